# revision 40
# baseline (speedup 1.0000x reference)
"""ChebyKAN Trainium2 kernel (partial-fp8 DoubleRow).

Reference computation:
    t = tanh(x)                      # x: [8192, 768]
    cheby[b,i,d] = T_d(t[b,i])       # Chebyshev polys, d = 0..8
    out[b,j] = sum_{i,d} cheby[b,i,d] * coefficients[i,j,d]

Strategy (data-parallel over batch across 8 cores):
  - Each core gets a 1024-row batch shard, transposed on host to xt [768, 1024]
    so the contraction dim (in_features) lands on SBUF partitions.
  - out.T[j, b] = sum_k coeffK[k, j] * chebyK[k, b]; K = 6*128 i-tiles x 8
    degrees (d=0 contributes a j-constant folded into a bias at PSUM drain).
  - The kernel is PE-streaming-bound (85% TensorMatrix busy at 144us in the
    all-bf16 version), so 20 of the 48 K-planes run as fp8e4m3 DoubleRow
    matmuls: one DR matmul contracts TWO 128-row K-planes in the SAME
    ~216ns instruction slot as one bf16 matmul (HW-measured issue spacing;
    two fp8 weights/cell) - a full 2x on converted planes.  Per (half, jt)
    the 48 bf16 matmuls become 28 bf16 + 10 DR = 38 slots.
    fp8 on 20 planes (d6/d7/d8 everywhere + d5 on the last two i-tiles)
    costs 1.892e-2 L2 error, HW-verified bit-identical to the host numpy
    simulation of the same quantization on the fixed inputs (gate 2e-2;
    all-fp8 measured 2.9e-2 which is over the gate, bf16 is 1.8e-3).
  - fp8 weights must sit in e4m3's normal range, so ALL weights (bf16 + fp8)
    are host-scaled by 2^18 and each PSUM drain applies x2^-18 while adding
    the d=0 bias (one fused op either engine).  Output is stored bf16 (halves
    the store bytes; adds ~0.1% quadrature error) and upcast to f32 on host.
  - DR pairs: (T6,T7) within each i-tile; T8 paired across adjacent i-tiles;
    (T5@it4, T5@it5) as the 10th pair.  Moving pair tiles are [128, 2*512]
    fp8 buffers whose halves are written directly by the Chebyshev-
    recurrence ops (T6=2T3^2-1 etc. write fp8 dst), then streamed via a
    rearranged [128, 2, 512] AP.
  - Load DMAs are issued on the Sync queue in NEED order (the first ~23us
    is inbound-transfer-bound at ~0.33GB/ms): it0's degree slices and the
    it0/it1 fp8 blocks first, later i-tile blocks split so each lands just
    ahead of its first matmul.  The GpSimd DMA ring is ~3x slower per
    descriptor - measured, do not use it for loads.
  - Startup: scalar activation tables (Tanh/Square) are pre-loaded while
    the first xt DMA is in flight; the it0 chain through T2b runs in
    256-col chunks; the first d1 matmul chunk's start=True zeroes the whole
    2KB PSUM bank (HW zero-region granularity) so the second chunk and all
    later matmuls accumulate with start=False.
  - Chebyshev tiles via product identities: T2=2t^2-1, T3=2tT2-t, T4=2T2^2-1,
    T5=2T2T3-t, T6=2T3^2-1, T7=2T3T4-t, T8=2T4^2-1.  Squares on the Scalar
    engine; x2-and-subtract fused in single Vector ops.  No GpSimd compute.
  - Two passes over batch halves of 512; per pass all 6 j-tiles accumulate in
    6 single-bank PSUM tiles.  The last i-tile block of each half runs
    jt-major so PSUM drains pipeline behind the remaining matmuls.
  - PE warmed with dummy matmuls so HAM un-throttles before real work.
"""

import sys

for _p in ("/opt/trn_rl_repo",):
    if _p not in sys.path:
        sys.path.insert(0, _p)

import numpy as np

import concourse.bass as bass
import concourse.mybir as mybir
import concourse.tile as tile
from concourse import bacc
from concourse import bass_utils
from concourse.tile import TileContext

F32 = mybir.dt.float32
BF16 = mybir.dt.bfloat16
F8 = mybir.dt.float8e4
AF = mybir.ActivationFunctionType
OP = mybir.AluOpType
DR = mybir.MatmulPerfMode.DoubleRow

B, I, J, D1 = 8192, 768, 768, 9  # batch, in_features, out_features, degree+1
NCORES = 8
BPC = B // NCORES      # 1024 batch rows per core
IT = I // 128          # 6 i-tiles
JT = J // 128          # 6 j-tiles
HB = 512               # half-batch (matmul N)
NBF = 5                # degrees 1..5 stay bf16
SCALE_BITS = 18
WSC = float(2.0 ** SCALE_BITS)
INV = float(2.0 ** -SCALE_BITS)

# fp8 DoubleRow pairs: p=0..5 -> (it=p,d6),(it=p,d7); p=6..8 -> (2k,d8),(2k+1,d8);
# p=9 -> (it4,d5),(it5,d5)
NPAIR = 10
# DRAM/SBUF position of each pair's plane block, ordered by first use
# (it0: p0; it1: p1,p6; it2: p2; it3: p3,p7; it4: p4; it5: p5,p8,p9).
PAIR_POS = {0: 0, 1: 1, 6: 2, 2: 3, 3: 4, 7: 5, 4: 6, 5: 7, 8: 8, 9: 9}

_CACHE = {}


def _build_nc():
    nc = bacc.Bacc("TRN2", target_bir_lowering=False, debug=False,
                   num_devices=NCORES)
    xt = nc.dram_tensor("xt", [I, BPC], F32, kind="ExternalInput").ap()
    # cbf[it, i, (d-1)*J + j]: bf16 degree-slices 1..5, scaled by 2^18
    cbf = nc.dram_tensor("cbf", [IT, 128, NBF * J], BF16,
                         kind="ExternalInput").ap()
    # cf8[i, q*J + j]: fp8 planes, q = 2*PAIR_POS[p]+s, in pair-NEED order
    # (p0, p1, p6 | p2, p3, p7, p4, p5, p8) so the first 6 planes can load
    # as an early small DMA and the rest later.
    cf8 = nc.dram_tensor("cf8", [128, 2 * NPAIR * J], F8,
                         kind="ExternalInput").ap()
    bias = nc.dram_tensor("bias", [128, JT], F32, kind="ExternalInput").ap()
    out = nc.dram_tensor("out", [J, BPC], BF16, kind="ExternalOutput").ap()

    with TileContext(nc) as tc:
        with (
            tc.tile_pool(name="xtp", bufs=1) as xt_pool,
            tc.tile_pool(name="work", bufs=3) as work,
            tc.tile_pool(name="coeffp", bufs=1) as coeff_pool,
            tc.tile_pool(name="f8p", bufs=2) as f8_pool,
            tc.tile_pool(name="outp", bufs=6) as out_pool,
            tc.tile_pool(name="biasp", bufs=1) as bias_pool,
            tc.tile_pool(name="psum", bufs=8, space="PSUM") as psum_pool,
        ):
            # PE warm-up scratch (zeroed; HAM un-throttles after sustained
            # matmul activity).
            warm = work.tile([128, HB], BF16, name="warm", tag="warm", bufs=1)
            nc.vector.memset(warm, 0.0)

            # Pre-load the scalar engine's Tanh/Square activation tables
            # while the first xt DMA is in flight (the first table load is
            # ~1.3us and otherwise lands on the startup critical path).
            # Reads the vector-memset warm tile so no extra dependency.
            actw = work.tile([128, 1], F32, name="actw", tag="actw", bufs=1)
            nc.scalar.activation(actw, warm[:, 0:1], AF.Tanh)
            nc.scalar.activation(actw, warm[:, 0:1], AF.Square)

            bias_all = bias_pool.tile([128, JT], F32, name="bias_all",
                                      tag="bias_all")

            # fp8 weights: one DMA, one resident tile.
            w8 = coeff_pool.tile([128, 2 * NPAIR * J], F8, name="w8",
                                 tag="w8")

            xt_tiles = [None] * IT
            cbf_tiles = [None] * IT
            c0_tiles = [None] * NBF
            deferred_h1 = []

            def bf_lhsT(it, dm1, jt):
                if it == 0:
                    return c0_tiles[dm1][:, jt * 128:(jt + 1) * 128]
                base = dm1 * J + jt * 128
                return cbf_tiles[it][:, base:base + 128]

            def f8_lhsT(p, jt):
                # [128, 2, 128] view: planes 2q, 2q+1 at column block jt
                q = PAIR_POS[p]
                blk = w8[:, 2 * q * J:(2 * q + 2) * J]
                return blk.rearrange("p (k j) -> p k j", k=2)[
                    :, :, jt * 128:(jt + 1) * 128]

            def drain(jt, ps, hs, last=False):
                js = slice(jt * 128, (jt + 1) * 128)
                ob = out_pool.tile([128, HB], BF16, name="ob", tag="ob")
                bcol = bias_all[:, jt:jt + 1]
                if last:
                    # Split the very last drain so its store starts half
                    # a tile earlier (the kernel ends on this DMA).
                    for c in range(2):
                        cs = slice(c * 256, (c + 1) * 256)
                        os_ = slice(hs.start + c * 256, hs.start + (c + 1) * 256)
                        nc.vector.tensor_scalar(ob[:, cs], ps[jt][:, cs],
                                                INV, bcol, OP.mult, OP.add)
                        nc.sync.dma_start(out[js, os_], ob[:, cs])
                elif jt % 2 == 0:
                    nc.scalar.activation(ob, ps[jt], AF.Identity,
                                         bias=bcol, scale=INV)
                    # scalar.dma_start's queue has ~7us dispatch latency
                    # (HW-measured Q_XIV); only safe mid-kernel.
                    eng = nc.scalar if hs.start == 0 else nc.sync
                    eng.dma_start(out[js, hs], ob)
                else:
                    nc.vector.tensor_scalar(ob, ps[jt], INV, bcol,
                                            OP.mult, OP.add)
                    nc.sync.dma_start(out[js, hs], ob)

            # fp8 moving pair buffers, one per pair per half (bufs=2 tags).
            def pair67(it):
                return f8_pool.tile([128, 2 * HB], F8, name=f"pr67_{it}",
                                    tag=f"pr67_{it}")

            def pair8(k):
                return f8_pool.tile([128, 2 * HB], F8, name=f"pr8_{k}",
                                    tag=f"pr8_{k}")

            def rhs3d(t):
                return t[:, 0:2 * HB].rearrange("p (k n) -> p k n", k=2)

            for half in range(2):
                hs = slice(half * HB, (half + 1) * HB)
                ps = [psum_pool.tile([128, HB], F32, name="ps", tag="ps")
                      for _ in range(JT)]
                if half == 0:
                    # dummy matmuls into ps[0]; overwritten by the real
                    # k==0 matmul (start=True clears has_written).  They
                    # fill the PE-idle window until the first tanh lands
                    # AND ramp the p-state so the first real matmuls run
                    # at full rate (mid-p-state costs ~2x per matmul).
                    for _ in range(4):
                        nc.tensor.matmul(ps[0], lhsT=warm[:, :128], rhs=warm,
                                         start=True, stop=True)

                pr67_tiles = [None] * IT
                pr8_tiles = [None] * 3
                pr5_tile = None

                if half == 0:
                    # All input loads on the Sync queue (the GpSimd DMA ring
                    # is ~3x slower per descriptor, HW-measured), issued in
                    # NEED order so each block's data lands just ahead of
                    # its first matmul.  Half-split xt tiles: tanh(half 0)
                    # waits only on its own 512 columns.
                    for it in range(IT):
                        ir = slice(it * 128, (it + 1) * 128)
                        xtt = xt_pool.tile([128, HB], F32, name=f"xt0_{it}",
                                           tag=f"xt0_{it}")
                        xt1 = xt_pool.tile([128, HB], F32, name=f"xt1_{it}",
                                           tag=f"xt1_{it}")
                        deferred_h1.append((xt1, xt[ir, HB:BPC]))
                        xt_tiles[it] = (xtt, xt1)
                    for dm1 in range(NBF):
                        c0_tiles[dm1] = coeff_pool.tile(
                            [128, J], BF16, name=f"c0d{dm1}",
                            tag=f"c0d{dm1}")
                    for it in range(1, IT):
                        cbf_tiles[it] = coeff_pool.tile(
                            [128, NBF * J], BF16, name=f"ctb{it}",
                            tag=f"ctb{it}")

                    def c0slice(dm1):
                        return (c0_tiles[dm1],
                                cbf[0][:, dm1 * J:(dm1 + 1) * J])

                    # Transfer-bound window: the queue drains ~0.33GB/ms, so
                    # every block is split/placed to land just ahead of its
                    # first use (it-block k starts at ~6+8.2k us).
                    issues = [
                        # tiny bias load first: absorbs the ~2us DMA-path
                        # wake-up latency so the first xt chunk rides a
                        # warm queue
                        (bias_all, bias),
                        (xt_tiles[0][0][:, 0:256], xt[0:128, 0:256]),
                        (xt_tiles[0][0][:, 256:HB], xt[0:128, 256:HB]),
                        c0slice(0), c0slice(1),
                        # fp8 pair blocks for it0/it1 (p0, p1, p6)
                        (w8[:, 0:6 * J], cf8[:, 0:6 * J]),
                        c0slice(2),
                        (xt_tiles[1][0], xt[128:256, 0:HB]),
                        # it1 degrees 1-2 ahead of it0's last slices
                        (cbf_tiles[1][:, 0:2 * J], cbf[1][:, 0:2 * J]),
                        c0slice(3), c0slice(4),
                        (cbf_tiles[1][:, 2 * J:], cbf[1][:, 2 * J:]),
                        (xt_tiles[2][0], xt[256:384, 0:HB]),
                        (cbf_tiles[2][:, 0:2 * J], cbf[2][:, 0:2 * J]),
                        # fp8 pair p2 (used at it2)
                        (w8[:, 6 * J:8 * J], cf8[:, 6 * J:8 * J]),
                        (cbf_tiles[2][:, 2 * J:], cbf[2][:, 2 * J:]),
                        (xt_tiles[3][0], xt[384:512, 0:HB]),
                        (cbf_tiles[3][:, 0:2 * J], cbf[3][:, 0:2 * J]),
                        # remaining fp8 pair blocks (it3+)
                        (w8[:, 8 * J:], cf8[:, 8 * J:]),
                        (cbf_tiles[3][:, 2 * J:], cbf[3][:, 2 * J:]),
                        # it4/it5 run d5 in fp8, so skip their d5 bf16 slice
                        (xt_tiles[4][0], xt[512:640, 0:HB]),
                        (cbf_tiles[4][:, 0:4 * J], cbf[4][:, 0:4 * J]),
                        (xt_tiles[5][0], xt[640:768, 0:HB]),
                        (cbf_tiles[5][:, 0:4 * J], cbf[5][:, 0:4 * J]),
                    ] + deferred_h1
                    for dst, src in issues:
                        nc.sync.dma_start(dst, src)

                for it in range(IT):

                    p67 = pair67(it)
                    pr67_tiles[it] = p67
                    if it % 2 == 0:
                        pr8_tiles[it // 2] = pair8(it // 2)
                    p8 = pr8_tiles[it // 2]
                    d8s = slice((it % 2) * HB, (it % 2 + 1) * HB)
                    if it == 4:
                        pr5_tile = f8_pool.tile([128, 2 * HB], F8,
                                                name="pr5", tag="pr5")
                    d5s = slice((it - 4) * HB, (it - 3) * HB)

                    # Recurrence inputs (t, T2, T3, T4) stay f32; bf16
                    # mirrors feed the PE for d=1..4.
                    t = work.tile([128, HB], F32, name="t", tag="t")
                    tb = work.tile([128, HB], BF16, name="tb", tag="tb")
                    sq = work.tile([128, HB], F32, name="sq2", tag="sq2",
                                   bufs=2)
                    T2 = work.tile([128, HB], F32, name="T2", tag="T2")
                    T2b = work.tile([128, HB], BF16, name="T2b", tag="T2b")
                    if half == 0 and it == 0:
                        # Startup critical path: run the whole chain through
                        # T2b in 256-col chunks so each op (and the d1/d2
                        # matmuls) waits only on its own chunk's ancestors.
                        xtt = xt_tiles[it][half]
                        for cs in (slice(0, 256), slice(256, HB)):
                            nc.scalar.activation(tb[:, cs], xtt[:, cs],
                                                 AF.Tanh)
                        for cs in (slice(0, 256), slice(256, HB)):
                            nc.scalar.activation(t[:, cs], xtt[:, cs],
                                                 AF.Tanh)
                            nc.scalar.activation(sq[:, cs], t[:, cs],
                                                 AF.Square)
                            nc.vector.tensor_scalar(T2[:, cs], sq[:, cs],
                                                    2.0, 1.0, OP.mult,
                                                    OP.subtract)
                            nc.vector.tensor_copy(T2b[:, cs], T2[:, cs])
                    else:
                        nc.scalar.activation(t, xt_tiles[it][half], AF.Tanh)
                        nc.vector.tensor_copy(tb, t)
                        # T2 = 2 t^2 - 1
                        nc.scalar.activation(sq, t, AF.Square)
                        nc.vector.tensor_scalar(T2, sq, 2.0, 1.0, OP.mult,
                                                OP.subtract)
                        nc.vector.tensor_copy(T2b, T2)
                    # T3 = 2 t T2 - t
                    P = work.tile([128, HB], F32, name="P3", tag="P3",
                                  bufs=2)
                    nc.vector.tensor_mul(P, t, T2)
                    T3 = work.tile([128, HB], F32, name="T3", tag="T3")
                    nc.vector.scalar_tensor_tensor(T3, P, 2.0, t, OP.mult,
                                                   OP.subtract)
                    T3b = work.tile([128, HB], BF16, name="T3b", tag="T3b")
                    if half == 0 and it == 0:
                        # Block 0 only: scalar queue still busy with squares;
                        # vector reaches here sooner.
                        nc.vector.tensor_copy(T3b, T3)
                    else:
                        nc.scalar.activation(T3b, T3, AF.Identity)
                    # T4 = 2 T2^2 - 1
                    sq = work.tile([128, HB], F32, name="sq4", tag="sq4",
                                   bufs=2)
                    nc.scalar.activation(sq, T2, AF.Square)
                    T4 = work.tile([128, HB], F32, name="T4", tag="T4")
                    nc.vector.tensor_scalar(T4, sq, 2.0, 1.0, OP.mult,
                                            OP.subtract)
                    T4b = work.tile([128, HB], BF16, name="T4b", tag="T4b")
                    if half == 0 and it == 0:
                        nc.vector.tensor_copy(T4b, T4)
                    else:
                        nc.scalar.activation(T4b, T4, AF.Identity)
                    # T5 = 2 T2 T3 - t  (bf16 for it0-3; it4/5 write the
                    # fp8 cross-it pair p9 instead)
                    P = work.tile([128, HB], F32, name="P5", tag="P5",
                                  bufs=2)
                    nc.vector.tensor_mul(P, T2, T3)
                    if it < 4:
                        T5 = work.tile([128, HB], BF16, name="T5", tag="T5")
                        nc.vector.scalar_tensor_tensor(T5, P, 2.0, t,
                                                       OP.mult, OP.subtract)
                    else:
                        T5 = None
                        nc.vector.scalar_tensor_tensor(pr5_tile[:, d5s], P,
                                                       2.0, t, OP.mult,
                                                       OP.subtract)
                    # T6 = 2 T3^2 - 1  -> fp8 pair plane 0
                    sq = work.tile([128, HB], F32, name="sq6", tag="sq6",
                                   bufs=2)
                    nc.scalar.activation(sq, T3, AF.Square)
                    nc.vector.tensor_scalar(p67[:, 0:HB], sq, 2.0, 1.0,
                                            OP.mult, OP.subtract)
                    # T7 = 2 T3 T4 - t  -> fp8 pair plane 1
                    P = work.tile([128, HB], F32, name="P7", tag="P7",
                                  bufs=2)
                    nc.vector.tensor_mul(P, T3, T4)
                    nc.vector.scalar_tensor_tensor(p67[:, HB:2 * HB], P, 2.0,
                                                   t, OP.mult, OP.subtract)
                    # T8 = 2 T4^2 - 1  -> fp8 cross-it pair plane it%2
                    sq = work.tile([128, HB], F32, name="sq8", tag="sq8",
                                   bufs=2)
                    nc.scalar.activation(sq, T4, AF.Square)
                    nc.vector.tensor_scalar(p8[:, d8s], sq, 2.0, 1.0,
                                            OP.mult, OP.subtract)

                    Ts = (tb, T2b, T3b, T4b) + ((T5,) if T5 is not None
                                                else ())
                    # DR pairs whose planes are complete at this it.  it0's
                    # own pair is deferred to it1: at it0 the cold vector
                    # cascade delivers T7 well after the bf16 matmuls drain;
                    # from it1 on the chain runs a block ahead.
                    if it == 0 or it == IT - 1:
                        drs = []
                    elif it == 1:
                        drs = [0, 1, 6]
                    else:
                        drs = [it]
                        if it % 2 == 1:
                            drs.append(6 + it // 2)

                    if it == IT - 1:
                        # Final it-block of the half: jt-major so drains
                        # pipeline behind the remaining matmuls.
                        for jt in range(JT):
                            for dm1, Td in enumerate(Ts):
                                nc.tensor.matmul(
                                    ps[jt], lhsT=bf_lhsT(it, dm1, jt),
                                    rhs=Td, start=False, stop=False)
                            nc.tensor.matmul(
                                ps[jt], lhsT=f8_lhsT(it, jt),
                                rhs=rhs3d(pr67_tiles[it]),
                                start=False, stop=False, perf_mode=DR)
                            nc.tensor.matmul(
                                ps[jt], lhsT=f8_lhsT(6 + it // 2, jt),
                                rhs=rhs3d(pr8_tiles[it // 2]),
                                start=False, stop=False, perf_mode=DR)
                            nc.tensor.matmul(
                                ps[jt], lhsT=f8_lhsT(9, jt),
                                rhs=rhs3d(pr5_tile),
                                start=False, stop=True, perf_mode=DR)
                            drain(jt, ps, hs,
                                  last=(half == 1 and jt == JT - 1))
                    else:
                        for dm1, Td in enumerate(Ts):
                            if it == 0 and dm1 < 2 and half == 0:
                                # Startup split: d1 chunk 0 with start=True
                                # zeroes the whole 2KB PSUM bank (the HW
                                # zero region), so later chunks accumulate
                                # with start=False into the zeroed rest.
                                for ci, cs in enumerate((slice(0, 256),
                                                         slice(256, HB))):
                                    for jt in range(JT):
                                        nc.tensor.matmul(
                                            ps[jt][:, cs],
                                            lhsT=bf_lhsT(it, dm1, jt),
                                            rhs=Td[:, cs],
                                            start=(dm1 == 0 and ci == 0),
                                            stop=False)
                                continue
                            for jt in range(JT):
                                nc.tensor.matmul(
                                    ps[jt], lhsT=bf_lhsT(it, dm1, jt),
                                    rhs=Td,
                                    start=(it == 0 and dm1 == 0),
                                    stop=False)
                        for p in drs:
                            rhs = rhs3d(pr67_tiles[p] if p < IT
                                        else pr8_tiles[p - 6])
                            for jt in range(JT):
                                nc.tensor.matmul(
                                    ps[jt], lhsT=f8_lhsT(p, jt), rhs=rhs,
                                    start=False, stop=False, perf_mode=DR)


    nc.compile()
    return nc


def _get_nc():
    if "nc" not in _CACHE:
        _CACHE["nc"] = _build_nc()
    return _CACHE["nc"]


def _prep_inputs(x, coefficients):
    bf16 = mybir.dt.np(BF16)
    f8 = mybir.dt.np(F8)
    x = np.asarray(x, dtype=np.float32)
    coefficients = np.asarray(coefficients, dtype=np.float32)
    xt_full = np.ascontiguousarray(x.T)  # [768, 8192]

    cr = coefficients.reshape(IT, 128, J, D1)
    # bf16 degrees 1..5, scaled: cbf[it, i, (d-1)*J + j]
    arr = np.transpose(cr[:, :, :, 1:NBF + 1], (0, 1, 3, 2)) * WSC
    cbf_in = np.ascontiguousarray(arr.reshape(IT, 128, NBF * J).astype(bf16))

    # fp8 planes at position q=2*PAIR_POS[p]+s:
    # p<6 -> (it=p, d=6+s); p=6..8 -> (it=2(p-6)+s, d=8); p=9 -> (it=4+s, d=5)
    cf8_in = np.empty((128, 2 * NPAIR * J), dtype=f8)
    for p in range(NPAIR):
        for s in range(2):
            if p < 6:
                it, d = p, 6 + s
            elif p < 9:
                it, d = 2 * (p - 6) + s, 8
            else:
                it, d = 4 + s, 5
            q = 2 * PAIR_POS[p] + s
            w = np.clip(cr[it, :, :, d].astype(np.float64) * WSC, -240, 240)
            cf8_in[:, q * J:(q + 1) * J] = w.astype(f8)

    bias_in = np.ascontiguousarray(
        coefficients[:, :, 0].sum(axis=0).astype(np.float32).reshape(JT, 128).T
    )

    in_maps = []
    for c in range(NCORES):
        xt_c = np.ascontiguousarray(xt_full[:, c * BPC:(c + 1) * BPC])
        in_maps.append({"xt": xt_c, "cbf": cbf_in, "cf8": cf8_in,
                        "bias": bias_in})
    return in_maps


def _run(x, coefficients, trace=False, **run_kwargs):
    nc = _get_nc()
    in_maps = _prep_inputs(x, coefficients)
    res = bass_utils.run_bass_kernel_spmd(
        nc, in_maps, core_ids=list(range(NCORES)), trace=trace, **run_kwargs
    )
    out_full = np.empty((B, J), dtype=np.float32)
    for c in range(NCORES):
        out_full[c * BPC:(c + 1) * BPC, :] = \
            res.results[c]["out"].astype(np.float32).T
    return out_full, res


def kernel(x, coefficients):
    out, _ = _run(x, coefficients, trace=False)
    return out


if __name__ == "__main__":
    rng = np.random.default_rng(0)
    x = rng.standard_normal((B, I), dtype=np.float32)
    std = 1.0 / (I * D1)
    coefficients = (std * rng.standard_normal((I, J, D1))).astype(np.float32)
    out = kernel(x, coefficients)
    print("out", out.shape, out.dtype, float(np.abs(out).mean()))
